# revision 2
# baseline (speedup 1.0000x reference)
"""RWKV-4 block (TimeMix + ChannelMix) on 8 Trainium2 NeuronCores.

Sharding: data-parallel over batch (B=8 -> one batch element per core); no
collectives.  Per core, activations are kept transposed ([channel, time]) so
the WKV recurrence maps onto the DVE's hardware linear scan
(tensor_tensor_scan along the free axis, fp32 state) and channel-wise mix
coefficients become per-partition scalars.  LayerNorms run in the natural
[time, channel] layout; PE transposes move between the two.  All GEMMs run
in bf16 (full PE rate, overlapped LDWEIGHTS); WKV arithmetic in fp32.

The reference's log-space-stabilized WKV is computed here in direct form:
  lam = exp(-exp(time_decay)), eu = exp(time_first)      (host)
  A_t = lam*A_{t-1} + exp(k_t)*v_t ;  B_t likewise with exp(k_t)
  y_t = (A_{t-1} + eu*exp(k_t)*v_t) / (B_{t-1} + eu*exp(k_t))
which is exact in infinite precision; with this problem's magnitudes the
fp32 accumulators stay in range (|B| < ~5e3) so no stabilization is needed.
"""

import os
import sys
from contextlib import ExitStack

import numpy as np

for _p in ("/opt/trn_rl_repo", "/root/.axon_site/_ro/trn_rl_repo"):
    if os.path.isdir(_p) and _p not in sys.path:
        sys.path.insert(0, _p)
        break

import concourse.bass as bass
import concourse.tile as tile
from concourse import mybir, bacc
from concourse.bass_utils import run_bass_kernel_spmd
from concourse.masks import make_identity

f32 = mybir.dt.float32
bf16 = mybir.dt.bfloat16
AF = mybir.ActivationFunctionType
ALU = mybir.AluOpType
P = 128
EPS = 1e-5
ts = bass.ts

B, T, C, DA, DF = 8, 2048, 1024, 1024, 4096
N_CORES = 8


def build_rwkv_kernel(nc, T=T, C=C, DA=DA, DF=DF, TT=512):
    n_ck = C // P
    n_dk = DA // P
    n_fk = DF // P
    n_t = T // TT
    su = min(C, 512)
    n_su = C // su
    n_rsub = TT // P
    assert C % P == 0 and DA % P == 0 and DF % P == 0 and T % TT == 0
    assert TT % P == 0 and C % su == 0

    dma = nc.sync.dma_start

    x_d = nc.dram_tensor("x", [T, C], f32, kind="ExternalInput")
    wkT_d = nc.dram_tensor("WkT", [C, DA], bf16, kind="ExternalInput")
    wvT_d = nc.dram_tensor("WvT", [C, DA], bf16, kind="ExternalInput")
    wrT_d = nc.dram_tensor("WrT", [C, DA], bf16, kind="ExternalInput")
    woT_d = nc.dram_tensor("WoT", [DA, C], bf16, kind="ExternalInput")
    fkT_d = nc.dram_tensor("FkT", [C, DF], bf16, kind="ExternalInput")
    fvT_d = nc.dram_tensor("FvT", [DF, C], bf16, kind="ExternalInput")
    frT_d = nc.dram_tensor("FrT", [C, C], bf16, kind="ExternalInput")
    vc_d = nc.dram_tensor("vecC", [P, 9 * n_ck], f32, kind="ExternalInput")
    vd_d = nc.dram_tensor("vecD", [P, 2 * n_dk], f32, kind="ExternalInput")
    out_d = nc.dram_tensor("out", [T, C], f32, kind="ExternalOutput")

    with tile.TileContext(nc) as tc, ExitStack() as top:
        const = top.enter_context(tc.tile_pool(name="const", bufs=1))
        vc = const.tile([P, 9, n_ck], f32)
        dma(out=vc, in_=vc_d[:].rearrange("p (r a) -> p r a", a=n_ck))
        vd = const.tile([P, 2, n_dk], f32)
        dma(out=vd, in_=vd_d[:].rearrange("p (r a) -> p r a", a=n_dk))
        V = {
            "ln1_g": lambda ck: vc[:, 0, ck:ck + 1],
            "ln1_b": lambda ck: vc[:, 1, ck:ck + 1],
            "ln2_g": lambda ck: vc[:, 2, ck:ck + 1],
            "ln2_b": lambda ck: vc[:, 3, ck:ck + 1],
            "tm_k": lambda ck: vc[:, 4, ck:ck + 1],
            "tm_v": lambda ck: vc[:, 5, ck:ck + 1],
            "tm_r": lambda ck: vc[:, 6, ck:ck + 1],
            "fm_k": lambda ck: vc[:, 7, ck:ck + 1],
            "fm_r": lambda ck: vc[:, 8, ck:ck + 1],
            "lam": lambda dk: vd[:, 0, dk:dk + 1],
            "eu": lambda dk: vd[:, 1, dk:dk + 1],
        }
        ident_b = const.tile([P, P], bf16)
        make_identity(nc, ident_b)
        ident_f = const.tile([P, P], f32)
        make_identity(nc, ident_f)
        eps_t = const.tile([P, 1], f32)
        nc.vector.memset(eps_t, EPS)
        carryA = const.tile([P, n_dk], f32)
        carryB = const.tile([P, n_dk], f32)

        dp_rwkv = top.enter_context(
            tc.tile_pool(name="dp_rwkv", bufs=n_dk * n_t, space="DRAM"))
        dp_gk = top.enter_context(
            tc.tile_pool(name="dp_gk", bufs=n_ck * n_t, space="DRAM"))
        dp_gr = top.enter_context(
            tc.tile_pool(name="dp_gr", bufs=n_ck * n_t, space="DRAM"))
        dp_out1 = top.enter_context(
            tc.tile_pool(name="dp_out1", bufs=T // P, space="DRAM"))
        dp_kv = top.enter_context(
            tc.tile_pool(name="dp_kv", bufs=n_ck * n_t, space="DRAM"))
        rwkv_dr, gk_dr, gr_dr, out1_dr, kv_dr = {}, {}, {}, {}, {}

        def layernorm(pool, tagp, xr):
            st = pool.tile([P, n_su, 6], f32, tag=f"{tagp}_st", name=f"{tagp}_st")
            for j in range(n_su):
                nc.vector.bn_stats(out=st[:, j, :], in_=xr[:, ts(j, su)])
            mv = pool.tile([P, 2], f32, tag=f"{tagp}_mv", name=f"{tagp}_mv")
            nc.vector.bn_aggr(out=mv, in_=st)
            sd = pool.tile([P, 1], f32, tag=f"{tagp}_sd", name=f"{tagp}_sd")
            nc.scalar.activation(out=sd, in_=mv[:, 1:2], func=AF.Sqrt,
                                 bias=eps_t[:, 0:1])
            rstd = pool.tile([P, 1], f32, tag=f"{tagp}_rstd", name=f"{tagp}_rstd")
            nc.vector.reciprocal(out=rstd, in_=sd)
            nbias = pool.tile([P, 1], f32, tag=f"{tagp}_nb", name=f"{tagp}_nb")
            nc.vector.tensor_tensor(out=nbias, in0=mv[:, 0:1], in1=rstd, op=ALU.mult)
            nc.vector.tensor_scalar_mul(out=nbias, in0=nbias, scalar1=-1.0)
            return rstd, nbias

        # ---------------- Phase AB1 ----------------
        with ExitStack() as ctx:
            wp = ctx.enter_context(tc.tile_pool(name="ab1_w", bufs=1))
            wk_sb = wp.tile([P, n_ck, DA], bf16)
            wv_sb = wp.tile([P, n_ck, DA], bf16)
            wr_sb = wp.tile([P, n_ck, DA], bf16)
            dma(out=wk_sb, in_=wkT_d[:].rearrange("(a p) d -> p a d", p=P))
            dma(out=wv_sb, in_=wvT_d[:].rearrange("(a p) d -> p a d", p=P))
            dma(out=wr_sb, in_=wrT_d[:].rearrange("(a p) d -> p a d", p=P))

            ab1 = ctx.enter_context(tc.tile_pool(name="ab1", bufs=2))
            mixp = ctx.enter_context(tc.tile_pool(name="ab1_mix", bufs=1))
            xp = ctx.enter_context(tc.tile_pool(name="ab1_x", bufs=4))
            wkv = ctx.enter_context(tc.tile_pool(name="wkv", bufs=2))
            ps_tr = ctx.enter_context(
                tc.tile_pool(name="ab1_ps_tr", bufs=2, space="PSUM"))
            ps_kvr = ctx.enter_context(
                tc.tile_pool(name="ab1_ps_kvr", bufs=2, space="PSUM"))

            prev_hT = None
            for it in range(n_t):
                ytile = []
                for rs in range(n_rsub):
                    xr = xp.tile([P, C], f32, tag="xr1", name="xr1")
                    dma(out=xr, in_=x_d[ts(it * n_rsub + rs, P), :])
                    rstd, nbias = layernorm(ab1, "l1", xr)
                    y = ab1.tile([P, C], bf16, tag=f"y{rs}", name=f"y{rs}")
                    nc.scalar.activation(out=y, in_=xr, func=AF.Identity,
                                         bias=nbias[:, 0:1], scale=rstd[:, 0:1])
                    ytile.append(y)

                hT = []
                for ck in range(n_ck):
                    pt = ps_tr.tile([P, TT], bf16, tag="pt", name="pt")
                    for rs in range(n_rsub):
                        nc.tensor.transpose(pt[:, ts(rs, P)],
                                            ytile[rs][:, ts(ck, P)], ident_b)
                    h = ab1.tile([P, 1 + TT], bf16, tag=f"hT{ck}", name=f"hT{ck}")
                    nc.scalar.activation(out=h[:, 1:1 + TT], in_=pt,
                                         func=AF.Identity,
                                         bias=V["ln1_b"](ck), scale=V["ln1_g"](ck))
                    if it == 0:
                        nc.vector.memset(h[:, 0:1], 0.0)
                    else:
                        nc.gpsimd.tensor_copy(out=h[:, 0:1],
                                              in_=prev_hT[ck][:, TT:TT + 1])
                    hT.append(h)

                xk, xv, xr_ = [], [], []
                for ck in range(n_ck):
                    cur = hT[ck][:, 1:1 + TT]
                    prv = hT[ck][:, 0:TT]
                    d = ab1.tile([P, TT], bf16, tag="dmix", name="dmix")
                    nc.vector.tensor_tensor(out=d, in0=cur, in1=prv, op=ALU.subtract)
                    for lst, coef, tg in ((xk, "tm_k", "xk"), (xv, "tm_v", "xv"),
                                          (xr_, "tm_r", "xr")):
                        a = mixp.tile([P, TT], bf16, tag=f"{tg}{ck}",
                                      name=f"{tg}{ck}")
                        nc.vector.scalar_tensor_tensor(
                            out=a, in0=d, scalar=V[coef](ck), in1=prv,
                            op0=ALU.mult, op1=ALU.add)
                        lst.append(a)

                for dk in range(n_dk):
                    pk = ps_kvr.tile([P, TT], f32, tag="pk", name="pk")
                    pv = ps_kvr.tile([P, TT], f32, tag="pv", name="pv")
                    pr = ps_kvr.tile([P, TT], f32, tag="pr", name="pr")
                    for ck in range(n_ck):
                        nc.tensor.matmul(pk, wk_sb[:, ck, ts(dk, P)], xk[ck],
                                         start=(ck == 0), stop=(ck == n_ck - 1))
                    for ck in range(n_ck):
                        nc.tensor.matmul(pv, wv_sb[:, ck, ts(dk, P)], xv[ck],
                                         start=(ck == 0), stop=(ck == n_ck - 1))
                    for ck in range(n_ck):
                        nc.tensor.matmul(pr, wr_sb[:, ck, ts(dk, P)], xr_[ck],
                                         start=(ck == 0), stop=(ck == n_ck - 1))

                    ek = wkv.tile([P, TT], f32, tag="ek", name="ek")
                    nc.scalar.activation(out=ek, in_=pk, func=AF.Exp)
                    er = wkv.tile([P, TT], f32, tag="er", name="er")
                    nc.scalar.activation(out=er, in_=pr, func=AF.Exp, scale=-1.0)
                    ekv = wkv.tile([P, TT], f32, tag="ekv", name="ekv")
                    nc.vector.tensor_tensor(out=ekv, in0=ek, in1=pv, op=ALU.mult)

                    A = wkv.tile([P, 1 + TT], f32, tag="A", name="A")
                    Bt = wkv.tile([P, 1 + TT], f32, tag="B", name="B")
                    lam_b = V["lam"](dk).to_broadcast([P, TT])
                    if it == 0:
                        nc.vector.memset(A[:, 0:1], 0.0)
                        nc.vector.memset(Bt[:, 0:1], 0.0)
                    else:
                        nc.gpsimd.tensor_copy(out=A[:, 0:1],
                                              in_=carryA[:, dk:dk + 1])
                        nc.gpsimd.tensor_copy(out=Bt[:, 0:1],
                                              in_=carryB[:, dk:dk + 1])
                    nc.vector.tensor_tensor_scan(
                        out=A[:, 1:1 + TT], data0=lam_b, data1=ekv,
                        initial=A[:, 0:1], op0=ALU.mult, op1=ALU.add)
                    nc.vector.tensor_tensor_scan(
                        out=Bt[:, 1:1 + TT], data0=lam_b, data1=ek,
                        initial=Bt[:, 0:1], op0=ALU.mult, op1=ALU.add)
                    if it != n_t - 1:
                        nc.gpsimd.tensor_copy(out=carryA[:, dk:dk + 1],
                                              in_=A[:, TT:TT + 1])
                        nc.gpsimd.tensor_copy(out=carryB[:, dk:dk + 1],
                                              in_=Bt[:, TT:TT + 1])

                    num = wkv.tile([P, TT], f32, tag="num", name="num")
                    nc.vector.scalar_tensor_tensor(
                        out=num, in0=ekv, scalar=V["eu"](dk), in1=A[:, 0:TT],
                        op0=ALU.mult, op1=ALU.add)
                    den = wkv.tile([P, TT], f32, tag="den", name="den")
                    nc.vector.scalar_tensor_tensor(
                        out=den, in0=ek, scalar=V["eu"](dk), in1=Bt[:, 0:TT],
                        op0=ALU.mult, op1=ALU.add)
                    rec = wkv.tile([P, TT], f32, tag="rec", name="rec")
                    nc.vector.reciprocal_approx_fast(out=rec, in_=den)
                    yv = wkv.tile([P, TT], f32, tag="yv", name="yv")
                    nc.vector.tensor_tensor(out=yv, in0=num, in1=rec, op=ALU.mult)
                    srd = wkv.tile([P, TT], f32, tag="srd", name="srd")
                    nc.vector.tensor_scalar_add(out=srd, in0=er, scalar1=1.0)
                    srr = wkv.tile([P, TT], f32, tag="srr", name="srr")
                    nc.vector.reciprocal_approx_fast(out=srr, in_=srd)
                    rw = wkv.tile([P, TT], bf16, tag="rw", name="rw")
                    nc.vector.tensor_tensor(out=rw, in0=yv, in1=srr, op=ALU.mult)

                    rd = dp_rwkv.tile([P, TT], bf16, tag="rwkv_dr", name="rwkv_dr")
                    dma(out=rd, in_=rw)
                    rwkv_dr[(dk, it)] = rd
                prev_hT = hT

        # ---------------- Phase AB2 ----------------
        with ExitStack() as ctx:
            wp = ctx.enter_context(tc.tile_pool(name="ab2_w", bufs=1))
            wo_sb = wp.tile([P, n_dk, C], bf16)
            dma(out=wo_sb, in_=woT_d[:].rearrange("(a p) c -> p a c", p=P))

            ab2 = ctx.enter_context(tc.tile_pool(name="ab2", bufs=2))
            xp2 = ctx.enter_context(tc.tile_pool(name="ab2_x", bufs=4))
            ps_wo = ctx.enter_context(
                tc.tile_pool(name="ab2_ps_wo", bufs=2, space="PSUM"))
            ps_o = ctx.enter_context(
                tc.tile_pool(name="ab2_ps_o", bufs=2, space="PSUM"))
            ps_g = ctx.enter_context(
                tc.tile_pool(name="ab2_ps_g", bufs=2, space="PSUM"))

            prev_gT = None
            for it in range(n_t):
                rws = []
                for dk in range(n_dk):
                    r = ab2.tile([P, TT], bf16, tag=f"rw2_{dk}", name=f"rw2_{dk}")
                    dma(out=r, in_=rwkv_dr[(dk, it)])
                    rws.append(r)
                xoT = []
                for ck in range(n_ck):
                    po = ps_wo.tile([P, TT], f32, tag="po", name="po")
                    for dk in range(n_dk):
                        nc.tensor.matmul(po, wo_sb[:, dk, ts(ck, P)], rws[dk],
                                         start=(dk == 0), stop=(dk == n_dk - 1))
                    xo = ab2.tile([P, TT], f32, tag=f"xoT{ck}", name=f"xoT{ck}")
                    nc.scalar.copy(out=xo, in_=po)
                    xoT.append(xo)
                yt2 = []
                for rs in range(n_rsub):
                    pso = ps_o.tile([P, C], f32, tag="pso", name="pso")
                    for ck in range(n_ck):
                        nc.tensor.transpose(pso[:, ts(ck, P)],
                                            xoT[ck][:, ts(rs, P)], ident_f)
                    xr = xp2.tile([P, C], f32, tag="xr2", name="xr2")
                    dma(out=xr, in_=x_d[ts(it * n_rsub + rs, P), :])
                    o1 = xp2.tile([P, C], f32, tag="o1", name="o1")
                    nc.vector.tensor_tensor(out=o1, in0=xr, in1=pso, op=ALU.add)
                    od = dp_out1.tile([P, C], f32, tag="out1_dr", name="out1_dr")
                    dma(out=od, in_=o1)
                    out1_dr[it * n_rsub + rs] = od
                    rstd, nbias = layernorm(ab2, "l2", o1)
                    y2 = ab2.tile([P, C], bf16, tag=f"y2_{rs}", name=f"y2_{rs}")
                    nc.scalar.activation(out=y2, in_=o1, func=AF.Identity,
                                         bias=nbias[:, 0:1], scale=rstd[:, 0:1])
                    yt2.append(y2)
                gT = []
                for ck in range(n_ck):
                    pg = ps_g.tile([P, TT], bf16, tag="pg", name="pg")
                    for rs in range(n_rsub):
                        nc.tensor.transpose(pg[:, ts(rs, P)],
                                            yt2[rs][:, ts(ck, P)], ident_b)
                    gt = ab2.tile([P, 1 + TT], bf16, tag=f"gT{ck}", name=f"gT{ck}")
                    nc.scalar.activation(out=gt[:, 1:1 + TT], in_=pg,
                                         func=AF.Identity,
                                         bias=V["ln2_b"](ck), scale=V["ln2_g"](ck))
                    if it == 0:
                        nc.vector.memset(gt[:, 0:1], 0.0)
                    else:
                        nc.gpsimd.tensor_copy(out=gt[:, 0:1],
                                              in_=prev_gT[ck][:, TT:TT + 1])
                    gT.append(gt)
                for ck in range(n_ck):
                    cur = gT[ck][:, 1:1 + TT]
                    prv = gT[ck][:, 0:TT]
                    d2 = ab2.tile([P, TT], bf16, tag="d2", name="d2")
                    nc.vector.tensor_tensor(out=d2, in0=cur, in1=prv,
                                            op=ALU.subtract)
                    gk = ab2.tile([P, TT], bf16, tag=f"gk{ck}", name=f"gk{ck}")
                    nc.vector.scalar_tensor_tensor(
                        out=gk, in0=d2, scalar=V["fm_k"](ck), in1=prv,
                        op0=ALU.mult, op1=ALU.add)
                    gr = ab2.tile([P, TT], bf16, tag=f"gr{ck}", name=f"gr{ck}")
                    nc.vector.scalar_tensor_tensor(
                        out=gr, in0=d2, scalar=V["fm_r"](ck), in1=prv,
                        op0=ALU.mult, op1=ALU.add)
                    gkd = dp_gk.tile([P, TT], bf16, tag="gk_dr", name="gk_dr")
                    dma(out=gkd, in_=gk)
                    gk_dr[(ck, it)] = gkd
                    grd = dp_gr.tile([P, TT], bf16, tag="gr_dr", name="gr_dr")
                    dma(out=grd, in_=gr)
                    gr_dr[(ck, it)] = grd
                prev_gT = gT

        # ---------------- Phase C ----------------
        with ExitStack() as ctx:
            wp = ctx.enter_context(tc.tile_pool(name="c_w", bufs=1))
            fk_sb = wp.tile([P, n_ck, DF], bf16)
            fv_sb = wp.tile([P, n_fk, C], bf16)
            dma(out=fk_sb, in_=fkT_d[:].rearrange("(a p) d -> p a d", p=P))
            dma(out=fv_sb, in_=fvT_d[:].rearrange("(a p) c -> p a c", p=P))

            cp = ctx.enter_context(tc.tile_pool(name="cp", bufs=2))
            gkp = ctx.enter_context(tc.tile_pool(name="c_gk", bufs=1))
            kfp = ctx.enter_context(tc.tile_pool(name="c_kf", bufs=1))
            ps_kf = ctx.enter_context(
                tc.tile_pool(name="c_ps_kf", bufs=2, space="PSUM"))
            ps_kv = ctx.enter_context(
                tc.tile_pool(name="c_ps_kv", bufs=2, space="PSUM"))

            n_half = 2 if n_fk > 8 else 1
            fph = n_fk // n_half
            for it in range(n_t):
                gks = []
                for ck in range(n_ck):
                    gk = gkp.tile([P, TT], bf16, tag=f"gkc{ck}", name=f"gkc{ck}")
                    dma(out=gk, in_=gk_dr[(ck, it)])
                    gks.append(gk)
                kf_h = [None] * n_half
                for hf in range(n_half):
                    kf_h[hf] = kfp.tile([P, fph, TT], bf16, tag=f"kf{hf}",
                                        name=f"kf{hf}")
                    for fj in range(fph):
                        fk = hf * fph + fj
                        pkf = ps_kf.tile([P, TT], f32, tag="pkf", name="pkf")
                        for ck in range(n_ck):
                            nc.tensor.matmul(pkf, fk_sb[:, ck, ts(fk, P)], gks[ck],
                                             start=(ck == 0), stop=(ck == n_ck - 1))
                        r1 = cp.tile([P, TT], f32, tag="r1", name="r1")
                        nc.scalar.activation(out=r1, in_=pkf, func=AF.Relu)
                        nc.vector.tensor_tensor(out=kf_h[hf][:, fj, :], in0=r1,
                                                in1=r1, op=ALU.mult)
                for ck in range(n_ck):
                    kvs = []
                    for hf in range(n_half):
                        pkv = ps_kv.tile([P, TT], f32, tag="pkv", name="pkv")
                        for fj in range(fph):
                            nc.tensor.matmul(pkv,
                                             fv_sb[:, hf * fph + fj, ts(ck, P)],
                                             kf_h[hf][:, fj, :],
                                             start=(fj == 0), stop=(fj == fph - 1))
                        kvs.append(pkv)
                    kv = cp.tile([P, TT], f32, tag="kv", name="kv")
                    if n_half == 1:
                        nc.scalar.copy(out=kv, in_=kvs[0])
                    else:
                        kv0 = cp.tile([P, TT], f32, tag="kv0", name="kv0")
                        nc.scalar.copy(out=kv0, in_=kvs[0])
                        nc.vector.tensor_tensor(out=kv, in0=kv0, in1=kvs[1],
                                                op=ALU.add)
                    kvd = dp_kv.tile([P, TT], f32, tag="kv_dr", name="kv_dr")
                    dma(out=kvd, in_=kv)
                    kv_dr[(ck, it)] = kvd

        # ---------------- Phase D ----------------
        with ExitStack() as ctx:
            wp = ctx.enter_context(tc.tile_pool(name="d_w", bufs=1))
            fr_sb = wp.tile([P, n_ck, C], bf16)
            dma(out=fr_sb, in_=frT_d[:].rearrange("(a p) c -> p a c", p=P))

            dpl = ctx.enter_context(tc.tile_pool(name="dpl", bufs=2))
            grp = ctx.enter_context(tc.tile_pool(name="d_gr", bufs=1))
            prp = ctx.enter_context(tc.tile_pool(name="d_pr", bufs=1))
            ps_rr = ctx.enter_context(
                tc.tile_pool(name="d_ps_rr", bufs=2, space="PSUM"))
            ps_pr = ctx.enter_context(
                tc.tile_pool(name="d_ps_pr", bufs=2, space="PSUM"))

            for it in range(n_t):
                grs = []
                for ck in range(n_ck):
                    gr = grp.tile([P, TT], bf16, tag=f"grd{ck}", name=f"grd{ck}")
                    dma(out=gr, in_=gr_dr[(ck, it)])
                    grs.append(gr)
                prods = []
                for ck in range(n_ck):
                    prr = ps_rr.tile([P, TT], f32, tag="prr", name="prr")
                    for cj in range(n_ck):
                        nc.tensor.matmul(prr, fr_sb[:, cj, ts(ck, P)], grs[cj],
                                         start=(cj == 0), stop=(cj == n_ck - 1))
                    sg = dpl.tile([P, TT], f32, tag="sg", name="sg")
                    nc.scalar.activation(out=sg, in_=prr, func=AF.Sigmoid)
                    kv = dpl.tile([P, TT], f32, tag="kvd", name="kvd")
                    dma(out=kv, in_=kv_dr[(ck, it)])
                    pr_ = prp.tile([P, TT], f32, tag=f"prod{ck}", name=f"prod{ck}")
                    nc.vector.tensor_tensor(out=pr_, in0=sg, in1=kv, op=ALU.mult)
                    prods.append(pr_)
                for rs in range(n_rsub):
                    psp = ps_pr.tile([P, C], f32, tag="psp", name="psp")
                    for ck in range(n_ck):
                        nc.tensor.transpose(psp[:, ts(ck, P)],
                                            prods[ck][:, ts(rs, P)], ident_f)
                    row = it * n_rsub + rs
                    o1 = dpl.tile([P, C], f32, tag="o1d", name="o1d")
                    dma(out=o1, in_=out1_dr[row])
                    fin = dpl.tile([P, C], f32, tag="fin", name="fin")
                    nc.vector.tensor_tensor(out=fin, in0=o1, in1=psp, op=ALU.add)
                    dma(out=out_d[ts(row, P), :], in_=fin)
    return nc


def make_host_inputs(inputs, C=C, DA=DA):
    import ml_dtypes
    bf = ml_dtypes.bfloat16
    a = np.asarray
    n_ck = C // P
    n_dk = DA // P
    vecC = np.stack([
        a(inputs["ln1_g"]), a(inputs["ln1_b"]),
        a(inputs["ln2_g"]), a(inputs["ln2_b"]),
        a(inputs["tm_k"]), a(inputs["tm_v"]), a(inputs["tm_r"]),
        a(inputs["fm_k"]), a(inputs["fm_r"]),
    ]).astype(np.float32)
    vecD = np.stack([
        np.exp(-np.exp(a(inputs["time_decay"]).astype(np.float64))),
        np.exp(a(inputs["time_first"]).astype(np.float64)),
    ]).astype(np.float32)
    vecC_pm = np.ascontiguousarray(
        vecC.reshape(9, n_ck, P).transpose(2, 0, 1).reshape(P, 9 * n_ck))
    vecD_pm = np.ascontiguousarray(
        vecD.reshape(2, n_dk, P).transpose(2, 0, 1).reshape(P, 2 * n_dk))
    t = lambda w: np.ascontiguousarray(a(w).astype(np.float32).T.astype(bf))
    return {
        "WkT": t(inputs["Wk"]), "WvT": t(inputs["Wv"]), "WrT": t(inputs["Wr"]),
        "WoT": t(inputs["Wo"]), "FkT": t(inputs["Fk"]), "FvT": t(inputs["Fv"]),
        "FrT": t(inputs["Fr"]), "vecC": vecC_pm, "vecD": vecD_pm,
    }


_NC = None
LAST_EXEC_NS = None


def _get_nc():
    global _NC
    if _NC is None:
        nc = bacc.Bacc("TRN2", target_bir_lowering=False, debug=False)
        build_rwkv_kernel(nc)
        nc.compile()
        _NC = nc
    return _NC


def _maybe_install_trace_hook():
    """Best-effort NTFF profile hook shim (used when RWKV_BASS_TRACE=1)."""
    import types
    try:
        from antenv.axon_hooks import get_axon_ntff_profile_hook  # noqa: F401
        return True
    except ImportError:
        pass
    try:
        if "/root/.axon_site" not in sys.path and os.path.isdir("/root/.axon_site"):
            sys.path.insert(0, "/root/.axon_site")
        from trn_agent_boot.trn_boot import _ntff_profile_via_ctypes
        import antenv
        hookmod = types.ModuleType("antenv.axon_hooks")
        hookmod._hook = _ntff_profile_via_ctypes("/opt/axon/libaxon_pjrt.so")
        hookmod.set_axon_ntff_profile_hook = lambda h: setattr(hookmod, "_hook", h)
        hookmod.get_axon_ntff_profile_hook = lambda: hookmod._hook
        sys.modules["antenv.axon_hooks"] = hookmod
        antenv.axon_hooks = hookmod
        return True
    except Exception:
        return False


def kernel(**inputs):
    global LAST_EXEC_NS
    x = np.asarray(inputs["x"], dtype=np.float32)
    assert x.shape == (B, T, C), x.shape
    nc = _get_nc()
    shared = make_host_inputs(inputs)
    in_maps = [dict(shared, x=np.ascontiguousarray(x[i])) for i in range(N_CORES)]
    trace = os.environ.get("RWKV_BASS_TRACE", "") == "1"
    if trace:
        trace = _maybe_install_trace_hook()
    res = run_bass_kernel_spmd(nc, in_maps, list(range(N_CORES)), trace=trace)
    LAST_EXEC_NS = res.exec_time_ns
    out = np.stack([res.results[i]["out"] for i in range(N_CORES)])
    return out.astype(np.float32)
